# revision 4
# baseline (speedup 1.0000x reference)
"""Multi-head attention kernel for Trainium2, sharded over 8 NeuronCores.

Problem: q,k,v [2, 32, 2048, 128] f32, mask [2, 1, 2048, 2048] bool.
  out = softmax(q @ k.T / sqrt(128), where(mask)) @ v

Sharding (data + head parallel): core c -> batch c//4, heads (c%4)*8..+8.
Each core computes 8 heads entirely locally.

Per-head device algorithm (T=S=2048, H=128):
  - mm1 computes S^T (s on partitions, t on free) so that the exp'd tiles are
    already in the [s, t] layout that mm2 (O^T = V^T @ P^T) wants as its
    streaming operand -> no on-device transposes of the attention matrix.
  - mm1: psum[s_tile, t_blk] = kT[:, s_tile].T @ qT[:, t_blk]   (float32r)
  - ACT: E = exp(SCALE * psum) -> bf16 SBUF tiles (scale fused into ACT)
  - DVE: E *= maskT tile (bf16 {0,1}; multiplicative mask == additive -inf
    mask because exp(min_f32 + x) == 0 in f32)
  - PE:  psum_o += V[s_tile].T @ E   (O^T unnormalized, accumulated over s)
  - PE:  psum_l += ones.T @ E        (softmax denominator l[t])
  - DVE evacuates psum -> SBUF (DMA cannot access PSUM on TRN2), DMA out.
Host divides O^T by l and transposes back to [t, h] while unsharding.
"""

import sys

try:
    import concourse  # noqa: F401
except ImportError:  # pragma: no cover
    sys.path.insert(0, "/opt/trn_rl_repo")

from contextlib import ExitStack

import numpy as np
import ml_dtypes

import concourse.bacc as bacc
import concourse.tile as tile
from concourse import mybir
from concourse.bass_utils import run_bass_kernel_spmd

N_CORES = 8
B, N, T, S, H = 2, 32, 2048, 2048, 128
HPC = 8  # heads per core
NS = S // 128  # 16 s-tiles
TB = 1024  # t block width (2 psum banks)
NTB = T // TB
MM_N = 512  # matmul free-dim (1 psum bank)
SCALE = 1.0 / np.sqrt(128.0)

_CACHE = {}


def _build():
    f32 = mybir.dt.float32
    f32r = mybir.dt.float32r
    bf16 = mybir.dt.bfloat16

    nc = bacc.Bacc("TRN2", target_bir_lowering=False, debug=False,
                   num_devices=N_CORES)

    qT = nc.dram_tensor("qT", [HPC, H, T], f32r, kind="ExternalInput").ap()
    kT = nc.dram_tensor("kT", [HPC, H, S], f32r, kind="ExternalInput").ap()
    v = nc.dram_tensor("v", [HPC, S, H], bf16, kind="ExternalInput").ap()
    mT = nc.dram_tensor("mT", [S, T], bf16, kind="ExternalInput").ap()
    oT = nc.dram_tensor("oT", [HPC, H, T], f32, kind="ExternalOutput").ap()
    lout = nc.dram_tensor("l", [HPC, T], f32, kind="ExternalOutput").ap()

    with tile.TileContext(nc) as tc, ExitStack() as ctx:
        consts = ctx.enter_context(tc.tile_pool(name="consts", bufs=1))
        qk = ctx.enter_context(tc.tile_pool(name="qk", bufs=2))
        vpool = ctx.enter_context(tc.tile_pool(name="vp", bufs=2))
        epool = ctx.enter_context(tc.tile_pool(name="e", bufs=6))
        osb = ctx.enter_context(tc.tile_pool(name="osb", bufs=2))
        ps_s = ctx.enter_context(tc.tile_pool(name="ps_s", bufs=2, space="PSUM"))
        ps_o = ctx.enter_context(tc.tile_pool(name="ps_o", bufs=1, space="PSUM"))
        ps_l = ctx.enter_context(tc.tile_pool(name="ps_l", bufs=1, space="PSUM"))

        # mask^T resident for the whole kernel: [128, s_tile, t]
        mask_sb = consts.tile([128, NS, T], bf16)
        nc.sync.dma_start(out=mask_sb, in_=mT.rearrange("(i p) t -> p i t", p=128))
        ones_sb = consts.tile([128, 1], bf16)
        nc.vector.memset(ones_sb, 1.0)

        for h in range(HPC):
            qT_sb = qk.tile([H, T], f32r, tag="q")
            nc.sync.dma_start(out=qT_sb, in_=qT[h])
            kT_sb = qk.tile([H, S], f32r, tag="k")
            nc.sync.dma_start(out=kT_sb, in_=kT[h])
            v_sb = vpool.tile([128, NS, H], bf16, tag="v")
            nc.sync.dma_start(out=v_sb, in_=v[h].rearrange("(i p) d -> p i d", p=128))

            for tb in range(NTB):
                tsl = slice(tb * TB, (tb + 1) * TB)
                po = ps_o.tile([H, TB], mybir.dt.float32, tag="po")
                pl = ps_l.tile([1, TB], mybir.dt.float32, tag="pl")
                for si in range(NS):
                    ps = ps_s.tile([128, TB], mybir.dt.float32, tag="ps")
                    ksl = kT_sb[:, si * 128:(si + 1) * 128]
                    for c in range(TB // MM_N):
                        nc.tensor.matmul(
                            ps[:, c * MM_N:(c + 1) * MM_N], ksl,
                            qT_sb[:, tb * TB + c * MM_N: tb * TB + (c + 1) * MM_N],
                            start=True, stop=True)
                    e = epool.tile([128, TB], bf16, tag="e")
                    nc.scalar.activation(e, ps, mybir.ActivationFunctionType.Exp,
                                         scale=SCALE)
                    nc.vector.tensor_mul(e, e, mask_sb[:, si, tsl])
                    for c in range(TB // MM_N):
                        csl = slice(c * MM_N, (c + 1) * MM_N)
                        nc.tensor.matmul(po[:, csl], v_sb[:, si, :], e[:, csl],
                                         start=(si == 0), stop=(si == NS - 1))
                        nc.tensor.matmul(pl[:, csl], ones_sb, e[:, csl],
                                         start=(si == 0), stop=(si == NS - 1))
                # evacuate psum -> SBUF (DMA has no PSUM port), then DMA out
                o_sb = osb.tile([H, TB], mybir.dt.float32, tag="o")
                nc.vector.tensor_copy(o_sb, po)
                nc.sync.dma_start(out=oT[h][:, tsl], in_=o_sb)
                l_sb = osb.tile([1, TB], mybir.dt.float32, tag="l")
                nc.vector.tensor_copy(l_sb, pl)
                nc.sync.dma_start(out=lout[h:h + 1, tsl], in_=l_sb)

    nc.compile()
    return nc


def _get_nc():
    if "nc" not in _CACHE:
        _CACHE["nc"] = _build()
    return _CACHE["nc"]


def _shard_inputs(q, k, v, mask):
    bf16 = ml_dtypes.bfloat16
    in_maps = []
    maskT = {}
    for b in range(B):
        maskT[b] = np.ascontiguousarray(mask[b, 0].T).astype(bf16)
    for c in range(N_CORES):
        b = c // 4
        h0 = (c % 4) * HPC
        in_maps.append({
            "qT": np.ascontiguousarray(
                q[b, h0:h0 + HPC].transpose(0, 2, 1)).astype(np.float32),
            "kT": np.ascontiguousarray(
                k[b, h0:h0 + HPC].transpose(0, 2, 1)).astype(np.float32),
            "v": v[b, h0:h0 + HPC].astype(bf16),
            "mT": maskT[b],
        })
    return in_maps


def kernel(q, k, v, mask):
    nc = _get_nc()
    in_maps = _shard_inputs(q, k, v, mask)
    res = run_bass_kernel_spmd(nc, in_maps, list(range(N_CORES)))
    out = np.empty((B, N, T, H), dtype=np.float32)
    for c in range(N_CORES):
        b = c // 4
        h0 = (c % 4) * HPC
        oT_c = res.results[c]["oT"]  # [HPC, H, T] unnormalized
        l_c = res.results[c]["l"]    # [HPC, T]
        out[b, h0:h0 + HPC] = (oT_c / l_c[:, None, :]).transpose(0, 2, 1)
    return out


# revision 6
# speedup vs baseline: 253.3174x; 253.3174x over previous
"""Multi-head attention kernel for Trainium2, sharded over 8 NeuronCores.

Problem: q,k,v [2, 32, 2048, 128] f32, mask [2, 1, 2048, 2048] bool.
  out = softmax(q @ k.T / sqrt(128), where(mask)) @ v

Sharding (data + head parallel): core c -> batch c//4, heads (c%4)*8..+8.
Each core computes 8 heads entirely locally.

Per-head device algorithm (T=S=2048, H=128):
  - mm1 computes S^T (s on partitions, t on free) so that the exp'd tiles are
    already in the [s, t] layout that mm2 (O^T = V^T @ P^T) wants as its
    streaming operand -> no on-device transposes of the attention matrix.
  - mm1: psum[s_tile, t_blk] = kT[:, s_tile].T @ qT[:, t_blk]   (float32r)
  - ACT: E = exp(SCALE * psum) -> bf16 SBUF tiles (scale fused into ACT)
  - DVE: E *= maskT tile (bf16 {0,1}; multiplicative mask == additive -inf
    mask because exp(min_f32 + x) == 0 in f32)
  - PE:  psum_o += V[s_tile].T @ E   (O^T unnormalized, accumulated over s)
  - PE:  psum_l += ones.T @ E        (softmax denominator l[t])
  - DVE evacuates psum -> SBUF (DMA cannot access PSUM on TRN2), DMA out.
Host divides O^T by l and transposes back to [t, h] while unsharding.
"""

import sys

try:
    import concourse  # noqa: F401
except ImportError:  # pragma: no cover
    sys.path.insert(0, "/opt/trn_rl_repo")

from contextlib import ExitStack

import numpy as np
import ml_dtypes

import concourse.bacc as bacc
import concourse.tile as tile
from concourse import mybir
from concourse.bass_utils import run_bass_kernel_spmd

N_CORES = 8
B, N, T, S, H = 2, 32, 2048, 2048, 128
HPC = 8  # heads per core
NS = S // 128  # 16 s-tiles
TB = 1024  # t block width (2 psum banks)
NTB = T // TB
MM_N = 512  # matmul free-dim (1 psum bank)
SCALE = 1.0 / np.sqrt(128.0)

_CACHE = {}


def _build(repeat=1):
    f32 = mybir.dt.float32
    f32r = mybir.dt.float32r
    bf16 = mybir.dt.bfloat16

    nc = bacc.Bacc("TRN2", target_bir_lowering=False, debug=False,
                   num_devices=N_CORES)

    qT = nc.dram_tensor("qT", [HPC, H, T], f32r, kind="ExternalInput").ap()
    kT = nc.dram_tensor("kT", [HPC, H, S], f32r, kind="ExternalInput").ap()
    v = nc.dram_tensor("v", [HPC, S, H], bf16, kind="ExternalInput").ap()
    mT = nc.dram_tensor("mT", [S, T], bf16, kind="ExternalInput").ap()
    oT = nc.dram_tensor("oT", [HPC, H, T], f32, kind="ExternalOutput").ap()
    lout = nc.dram_tensor("l", [HPC, T], f32, kind="ExternalOutput").ap()

    with tile.TileContext(nc) as tc, ExitStack() as ctx:
        consts = ctx.enter_context(tc.tile_pool(name="consts", bufs=1))
        qk = ctx.enter_context(tc.tile_pool(name="qk", bufs=2))
        vpool = ctx.enter_context(tc.tile_pool(name="vp", bufs=2))
        epool = ctx.enter_context(tc.tile_pool(name="e", bufs=6))
        osb = ctx.enter_context(tc.tile_pool(name="osb", bufs=2))
        ps_s = ctx.enter_context(tc.tile_pool(name="ps_s", bufs=2, space="PSUM"))
        ps_o = ctx.enter_context(tc.tile_pool(name="ps_o", bufs=1, space="PSUM"))
        ps_l = ctx.enter_context(tc.tile_pool(name="ps_l", bufs=1, space="PSUM"))

        # mask^T resident for the whole kernel: [128, s_tile, t]
        mask_sb = consts.tile([128, NS, T], bf16)
        nc.sync.dma_start(out=mask_sb, in_=mT.rearrange("(i p) t -> p i t", p=128))
        ones_sb = consts.tile([128, 1], bf16)
        nc.vector.memset(ones_sb, 1.0)

        rep_ctx = tc.For_i(0, repeat, 1) if repeat > 1 else None
        if rep_ctx is not None:
            ctx.enter_context(rep_ctx)

        for h in range(HPC):
            qT_sb = qk.tile([H, T], f32r, tag="q")
            nc.sync.dma_start(out=qT_sb, in_=qT[h])
            kT_sb = qk.tile([H, S], f32r, tag="k")
            nc.sync.dma_start(out=kT_sb, in_=kT[h])
            v_sb = vpool.tile([128, NS, H], bf16, tag="v")
            nc.sync.dma_start(out=v_sb, in_=v[h].rearrange("(i p) d -> p i d", p=128))

            for tb in range(NTB):
                tsl = slice(tb * TB, (tb + 1) * TB)
                po = ps_o.tile([H, TB], mybir.dt.float32, tag="po")
                pl = ps_l.tile([1, TB], mybir.dt.float32, tag="pl")
                for si in range(NS):
                    ps = ps_s.tile([128, TB], mybir.dt.float32, tag="ps")
                    ksl = kT_sb[:, si * 128:(si + 1) * 128]
                    for c in range(TB // MM_N):
                        nc.tensor.matmul(
                            ps[:, c * MM_N:(c + 1) * MM_N], ksl,
                            qT_sb[:, tb * TB + c * MM_N: tb * TB + (c + 1) * MM_N],
                            start=True, stop=True)
                    e = epool.tile([128, TB], bf16, tag="e")
                    nc.scalar.activation(e, ps, mybir.ActivationFunctionType.Exp,
                                         scale=SCALE)
                    nc.vector.tensor_mul(e, e, mask_sb[:, si, tsl])
                    for c in range(TB // MM_N):
                        csl = slice(c * MM_N, (c + 1) * MM_N)
                        nc.tensor.matmul(po[:, csl], v_sb[:, si, :], e[:, csl],
                                         start=(si == 0), stop=(si == NS - 1))
                        nc.tensor.matmul(pl[:, csl], ones_sb, e[:, csl],
                                         start=(si == 0), stop=(si == NS - 1))
                # evacuate psum -> SBUF (DMA has no PSUM port), then DMA out
                o_sb = osb.tile([H, TB], mybir.dt.float32, tag="o")
                nc.vector.tensor_copy(o_sb, po)
                nc.sync.dma_start(out=oT[h][:, tsl], in_=o_sb)
                l_sb = osb.tile([1, TB], mybir.dt.float32, tag="l")
                nc.vector.tensor_copy(l_sb, pl)
                nc.sync.dma_start(out=lout[h:h + 1, tsl], in_=l_sb)

    nc.compile()
    return nc


def _get_nc():
    if "nc" not in _CACHE:
        _CACHE["nc"] = _build()
    return _CACHE["nc"]


def _shard_inputs(q, k, v, mask):
    bf16 = ml_dtypes.bfloat16
    in_maps = []
    maskT = {}
    for b in range(B):
        maskT[b] = np.ascontiguousarray(mask[b, 0].T).astype(bf16)
    for c in range(N_CORES):
        b = c // 4
        h0 = (c % 4) * HPC
        in_maps.append({
            "qT": np.ascontiguousarray(
                q[b, h0:h0 + HPC].transpose(0, 2, 1)).astype(np.float32),
            "kT": np.ascontiguousarray(
                k[b, h0:h0 + HPC].transpose(0, 2, 1)).astype(np.float32),
            "v": v[b, h0:h0 + HPC].astype(bf16),
            "mT": maskT[b],
        })
    return in_maps


def kernel(q, k, v, mask):
    nc = _get_nc()
    in_maps = _shard_inputs(q, k, v, mask)
    res = run_bass_kernel_spmd(nc, in_maps, list(range(N_CORES)))
    out = np.empty((B, N, T, H), dtype=np.float32)
    for c in range(N_CORES):
        b = c // 4
        h0 = (c % 4) * HPC
        oT_c = res.results[c]["oT"]  # [HPC, H, T] unnormalized
        l_c = res.results[c]["l"]    # [HPC, T]
        out[b, h0:h0 + HPC] = (oT_c / l_c[:, None, :]).transpose(0, 2, 1)
    return out
